# revision 96
# baseline (speedup 1.0000x reference)
"""AttnBlock (GroupNorm -> QKV 1x1 -> full NxN attention -> out-proj + residual)
on 8 Trainium2 NeuronCores, data-parallel over batch (1 batch element/core).

Shapes (hardcoded): x [8, 256, 64, 64] f32, c=256, n=h*w=4096.

Per-core scheme (all on-chip, no transposes):
  - x, q, k live in [c, n] layout: SBUF [128 part, 2 c-chunks, 4096].
    Matmuls read x via a float32r bitcast (no separate f32r copy of x).
  - GroupNorm's affine is folded host-side-style into the projection
    weights on-chip: A = rstd*gn_scale folds into w columns, the w^T@B
    terms fold into per-channel eviction biases (bq3/bk3) or ride the
    residual bias bo3 (softmax rows sum to 1).
  - scores computed transposed: sT[j, i] = sum_c k[c,j] q[c,i] (lhsT=k,
    rhs=q), softmax without max-subtraction (scores ~N(0,1), the 1/16
    scale folded into wq), exp on ScalarE PSUM->SBUF (2 j-tiles per op).
  - v is produced pre-transposed with the output projection folded in:
    v'T[j, co] = sum_ci x[ci, j] * (wo@wv)'[ci, co].
  - ib0's attention is interleaved with the k/v' projection n-blocks so
    the PE has no idle phase: per n-block, emit k-proj, v'-proj, then
    scores for the two j-groups that n-block just made available, with
    AV accumulation lagging two groups behind. Block 0's projection PSUM
    tiles borrow the still-idle score/ob tags (3-deep rotation) and the
    f32r x-rounds are prefetched 3 blocks ahead, so the projection
    stream never stalls on PSUM eviction or DVE rounding.
  - in the attention loop, scores(g+2) are emitted BEFORE av(g): the exp
    pipeline leads the AV consumer by a full AV-step, which removes the
    Activation-queue backlog stall at every i-block boundary.
  - out[co, i] = sum_j v'T[j, co] * exp(sT)[j, i] accumulates in PSUM;
    softmax denominators via partial sums of exp tiles (Pool groups 0-7,
    DVE groups 8-15) merged on DVE and folded by a single ones[128,128]
    f32r matmul per i-block (the last i-block keeps partially-direct
    folds and emits them BEFORE its final AV group, so the denominator
    lands ~0.4us earlier); division by the denominator fused with the
    residual + bias at eviction (scalar_tensor_tensor; the last i-block
    uses a Pool-precomputed x+bias tensor, a co1-first AV finish, and
    co0-full + co1-half output DMAs to minimize the serialized
    HWDGE/DMA/drain epilogue).
"""

import sys

if "/opt/trn_rl_repo" not in sys.path:
    sys.path.insert(0, "/opt/trn_rl_repo")

import numpy as np

P = 128
C = 256
CC = C // P          # 2 channel chunks
H = W = 64
N = H * W            # 4096
NJT = N // P         # 32 j-tiles of 128
IB = 512             # i-block (psum bank width)
NIB = N // IB        # 8 i-blocks
JG = 2               # j-tiles per exp group
NGRP = NJT // JG     # exp groups per i-block (16)
XCHS = [512] * 7 + [128] * 4   # x DMA / GN chunk sizes (small tail chunks
XCH = len(XCHS)                # so the last bn_stats lands right after the
G = 8                          # last data)
EPS = 1e-6
NWARM = 71           # PE warmup dummy matmuls riding the x DMA
NWARM2 = 0           # fillers riding the GN grp-chain latency
NWARM3 = 0           # fillers riding the ab/fold latency

_CACHE = {}


def _build():
    import concourse.tile as tile
    from concourse import bacc, mybir
    from concourse.bass_interp import get_hw_module

    f32 = mybir.dt.float32
    f32r = mybir.dt.float32r
    AF = mybir.ActivationFunctionType
    AX = mybir.AxisListType
    OP = mybir.AluOpType

    nc = bacc.Bacc("TRN2", target_bir_lowering=False, debug=False,
                   enable_asserts=False, num_devices=1)

    x_d = nc.dram_tensor("x", (C, N), f32, kind="ExternalInput").ap()
    ws_d = nc.dram_tensor("wstack", (3, C, C), f32, kind="ExternalInput").ap()
    bs_d = nc.dram_tensor("bstack", (5, C), f32, kind="ExternalInput").ap()
    g_d = nc.dram_tensor("Gm", (CC, P, G), f32, kind="ExternalInput").ap()
    gt_d = nc.dram_tensor("GmT", (CC, G, P), f32, kind="ExternalInput").ap()
    out_d = nc.dram_tensor("out", (C, N), f32, kind="ExternalOutput").ap()

    x_r = x_d.rearrange("(cc p) n -> p cc n", p=P)
    out_r = out_d.rearrange("(cc p) n -> p cc n", p=P)

    # the one ACT table set covering every func we use (ln, exp, copy, identity)
    from concourse.hw_specs import get_activation_tables
    act_sets = list(get_activation_tables(nc.m.arch))
    LNEXP_SET = act_sets.index("natural_log_exp_and_others")

    with tile.TileContext(nc) as tc:
        with (
            tc.tile_pool(name="const", bufs=1) as const,
            tc.tile_pool(name="data", bufs=1) as data,
            tc.tile_pool(name="work", bufs=1) as work,
            tc.tile_pool(name="ps", bufs=1, space="PSUM") as ps,
        ):
            _ld = mybir.InstLoadActFuncSet(
                name=nc.get_next_instruction_name(), ins=[], outs=[],
                act_func_set_id=LNEXP_SET)
            nc.scalar.add_instruction(_ld)

            xt = data.tile([P, CC, N], f32, tag="x")
            q_t = data.tile([P, CC, N], f32r, tag="q")
            k_t = data.tile([P, CC, N], f32r, tag="k")
            vp_t = data.tile([P, NJT, C], f32r, tag="vp")

            # ---- tiny constants on the ScalarE DMA queue so the x stream
            # owns the sync queue from t=0 ----
            b_t = const.tile([P, 5, CC], f32, tag="bt")
            nc.scalar.dma_start(b_t[:], bs_d.rearrange("v (cc p) -> p v cc", p=P))
            g_t = const.tile([P, CC, G], f32, tag="G")
            nc.scalar.dma_start(g_t[:], g_d.rearrange("cc p g -> p cc g"))
            gt_t = const.tile([G, CC, P], f32, tag="GT")
            nc.scalar.dma_start(gt_t[:], gt_d.rearrange("cc g p -> g cc p"))

            # ones on DVE (idle until the first bn_stats): the warm matmuls
            # below start as soon as ones_t lands
            ones_l = work.tile([P, P], f32, tag="onesl")
            nc.vector.memset(ones_l[:], 1.0)
            ones_t = const.tile([P, P], f32r, tag="ones")
            nc.vector.tensor_copy(ones_t[:], ones_l[:])
            eps_t = const.tile([G, 1], f32, tag="eps")
            nc.gpsimd.memset(eps_t[:], EPS)

            # ---- x DMA in chunks, bn_stats per chunk on DVE; dummy
            # matmuls on ones keep the PE p-state warm through the DMA ----
            st6 = work.tile([P, CC, XCH, 6], f32, tag="st6")
            ws_r = ws_d.rearrange("w (cc p) o -> p w cc o", p=P)
            w_l = work.tile([P, 3, CC, C], f32, tag="wl")
            xbs = {}
            xoff = 0
            for xc, csz in enumerate(XCHS):
                nsl = slice(xoff, xoff + csz)
                xoff += csz
                nc.sync.dma_start(xt[:, :, nsl], x_r[:, :, nsl])
                for cc in range(CC):
                    nc.vector.bn_stats(st6[:, cc, xc], xt[:, cc, nsl])
                if xc < 3 and csz == IB:
                    # pre-round the first n-blocks to f32r during the DMA
                    # window on ScalarE, unblocking the first projections
                    xb = work.tile([P, CC, IB], f32r, tag="xr", bufs=3,
                                   name="xr")
                    nc.scalar.activation(xb[:], xt[:, :, nsl], AF.Copy)
                    xbs[xc] = xb


            def warm(n):
                for wi in range(n):
                    wp = ps.tile([P, IB], f32, tag="qk", bufs=2, name="warm")
                    nc.tensor.matmul(wp[:, 0:P], ones_t[:], ones_t[:],
                                     start=True, stop=True)

            warm(NWARM)

            # ---- weights (queued after x) ----
            nc.sync.dma_start(w_l[:], ws_r)

            w_r = const.tile([P, 3, CC, C], f32r, tag="wr")
            wq_t, wk_t, wov_t = w_r[:, 0], w_r[:, 1], w_r[:, 2]
            bq_t, bk_t, bo_t, gns_t, gnb_t = (b_t[:, v] for v in range(5))

            # per-channel (mean, E[x^2]) from aggregated bn records
            stc = const.tile([P, CC, 2], f32, tag="stc")
            for cc in range(CC):
                nc.vector.bn_aggr(stc[:, cc], st6[:, cc])
                # (mean, var) -> (mean, E[x^2]) in place: mean*mean + var
                nc.vector.scalar_tensor_tensor(
                    stc[:, cc, 1:2], stc[:, cc, 0:1], stc[:, cc, 0:1],
                    stc[:, cc, 1:2], OP.mult, OP.add)

            # group-reduce per-channel (mean, E[x^2]) straight in PSUM
            gps = ps.tile([G, 2], f32, tag="qk", bufs=2, name="gps")
            for cc in range(CC):
                nc.tensor.matmul(gps[:], g_t[:, cc], stc[:, cc],
                                 start=(cc == 0), stop=(cc == CC - 1))

            # filler warmups: keep the PE queue busy while the grp chain
            # (DVE ops + ACT ln/exp) resolves, so chps dispatches hot
            warm(NWARM2)

            # grp cols: 0=mean 1=rstd 2=ex2 3=mean^2 4=var 5=ln(var+eps)
            CPG = C // G
            grp = const.tile([G, 6], f32, tag="grp")
            nc.vector.tensor_scalar_mul(grp[:, 0:1], gps[:, 0:1], 1.0 / CPG)
            nc.vector.tensor_scalar_mul(grp[:, 2:3], gps[:, 1:2], 1.0 / CPG)
            nc.vector.tensor_mul(grp[:, 3:4], grp[:, 0:1], grp[:, 0:1])
            nc.vector.tensor_sub(grp[:, 4:5], grp[:, 2:3], grp[:, 3:4])
            # rstd = exp(-0.5*ln(var+eps)): stays in the natural_log_exp set
            nc.scalar.activation(grp[:, 5:6], grp[:, 4:5], AF.Ln, bias=eps_t[:])
            nc.scalar.activation(grp[:, 1:2], grp[:, 5:6], AF.Exp, scale=-0.5)

            # broadcast group (mean, rstd) to channels; affine coeffs
            # A = rstd*gn_scale and B' = mean*A - gn_bias (= -B, so the
            # bias folds below subtract instead of add)
            # gn_scale is pre-folded into the weight columns and w^T@gn_bias
            # into the biases HOST-side, so on-chip A = rstd and B-term =
            # mean*rstd: the critical wq fold reads rstd straight off the
            # chps PSUM as its per-partition scalar (one DVE hop fewer
            # before the first projection). ab keeps SBUF copies for the
            # Pool folds (no PSUM access) and the bias matmuls.
            ab = const.tile([P, CC, 2], f32, tag="ab")  # 0=rstd 1=mean*rstd
            for cc in range(CC):
                chps = ps.tile([P, 2], f32, tag="qk", bufs=2, name="chps")
                nc.tensor.matmul(chps[:], gt_t[:, cc], grp[:, 0:2],
                                 start=True, stop=True)
                nc.vector.tensor_copy(ab[:, cc, 0:1], chps[:, 1:2])
                nc.vector.tensor_mul(ab[:, cc, 1:2], chps[:, 0:1],
                                     ab[:, cc, 0:1])

            # fold rstd into the weights (q first -- q(ib0) runs first)
            bq3 = const.tile([P, CC], f32, tag="bq3")
            bk3 = const.tile([P, CC], f32, tag="bk3")
            bo3 = const.tile([P, CC], f32, tag="bo3")
            for w in (0, 1, 2):
                for cc in range(CC):
                    # wq on DVE, wk/wov on Pool: the folds run in parallel
                    eng = nc.vector if w == 0 else nc.gpsimd
                    eng.tensor_scalar_mul(w_r[:, w, cc], w_l[:, w, cc],
                                          ab[:, cc, 0:1])

            def emit_bias(w, b_in, b_out):
                # fold the -w^T @ B' term into an eviction bias (tiny
                # matmuls; emitted after the first users' projection matmuls
                # so the DVE subs overlap them)
                for oc in range(CC):
                    bp = ps.tile([P, 1], f32, tag="ob", bufs=2, name="bp")
                    for ci in range(CC):
                        nc.tensor.matmul(bp[:],
                                         w_l[:, w, ci, oc * P:(oc + 1) * P],
                                         ab[:, ci, 1:2],
                                         start=(ci == 0), stop=(ci == CC - 1))
                    nc.vector.tensor_sub(b_out[:, oc:oc + 1],
                                         b_in[:, oc:oc + 1], bp[:])

            # rotating rounded copy of x: matmul operands must be rounded
            # to f32r by a compute engine (DMA output is raw f32, and the
            # raw x must be kept for the residual). Three n-blocks deep.
            def round_x(nb):
                # round an n-block of x to f32r (DVE 2x dual-port copy)
                nsl = slice(nb * IB, (nb + 1) * IB)
                xb = work.tile([P, CC, IB], f32r, tag="xr", bufs=3,
                               name="xr")
                nc.vector.tensor_copy(xb[:], xt[:, :, nsl])
                xbs[nb] = xb

            def proj_block(nb):
                # nb 0-2 were pre-rounded during the load; each block
                # prefetches the round for nb+3 (bufs=3 rotation) so the
                # first projection matmul never waits on DVE
                nsl = slice(nb * IB, (nb + 1) * IB)
                xb = xbs.pop(nb)
                # block 0 runs before any attention matmuls exist to
                # interleave with, so its PSUM tiles round-robin through the
                # still-unused score/ob tags (3-deep rotation) -- otherwise
                # the 2-buffer qk rotation stalls the PE on every eviction
                if nb <= 1:
                    tags = ["qk", "score", "ob"]
                elif nb == 2:
                    tags = ["qk", "ob"]
                else:
                    tags = ["qk"]
                tagi = [0]

                def ptile(width):
                    t = tags[tagi[0] % len(tags)]
                    tagi[0] += 1
                    return ps.tile([P, width], f32, tag=t, bufs=2,
                                   name="pq" if t == "qk" else f"pq_{t}")

                # q projection first: wq's fold sits on DVE right after the
                # ab computation (no cross-engine hop on the critical path)
                for oc in range(CC):
                    pq = ptile(IB)
                    for ci in range(CC):
                        nc.tensor.matmul(
                            pq[:], wq_t[:, ci, oc * P:(oc + 1) * P],
                            xb[:, ci],
                            start=(ci == 0), stop=(ci == CC - 1))
                    if oc == 0:
                        nc.vector.tensor_scalar_add(
                            q_t[:, oc, nsl], pq[:], bq3[:, oc:oc + 1])
                    else:
                        nc.scalar.activation(
                            q_t[:, oc, nsl], pq[:], AF.Identity,
                            bias=bq3[:, oc:oc + 1])
                # k projection (evictions on ScalarE)
                for oc in range(CC):
                    pp = ptile(IB)
                    for ci in range(CC):
                        nc.tensor.matmul(
                            pp[:], wk_t[:, ci, oc * P:(oc + 1) * P],
                            xb[:, ci],
                            start=(ci == 0), stop=(ci == CC - 1))
                    nc.scalar.activation(k_t[:, oc, nsl], pp[:], AF.Identity,
                                         bias=bk3[:, oc:oc + 1])
                # v'T for the 4 j-tiles of this n-block (evictions on DVE)
                # v' j-tiles in PAIRS: two disjoint 256-column accumulation
                # regions share one [P,512] PSUM tile, halving the rotation
                # slots and evictions (4 -> 2 per block)
                for jj2 in range(2):
                    pv = ptile(IB)
                    for jj in (2 * jj2, 2 * jj2 + 1):
                        off = (jj & 1) * C
                        for ci in range(CC):
                            nc.tensor.matmul(
                                pv[:, off:off + C],
                                xb[:, ci, jj * P:(jj + 1) * P],
                                wov_t[:, ci, :],
                                start=(ci == 0), stop=(ci == CC - 1))
                    jt0 = nb * 4 + 2 * jj2
                    nc.vector.tensor_copy(vp_t[:, jt0:jt0 + 2], pv[:])
                # prefetch the f32r round for block nb+3 (its xr buffer is
                # the one this block's matmuls just finished reading), two
                # block-periods before proj_block(nb+3) needs it
                if nb + 3 < NIB:
                    round_x(nb + 3)

            # filler warmups covering the ab/fold DVE latency before the
            # bias matmuls and first q projection dispatch
            warm(NWARM3)

            emit_bias(0, bq_t, bq3)
            emit_bias(1, bk_t, bk3)
            emit_bias(2, bo_t, bo3)

            # ---- attention: software-pipelined across i-blocks (the next
            # block's first two score groups are emitted during the current
            # block's last two iterations, so the exp pipeline never drains
            # at a block boundary) ----
            ssgs, ets, obd, esd, smtd = {}, {}, {}, {}, {}

            def scores(ib, g):
                # projections run one n-block AHEAD of the scores that
                # consume their k/v tiles, so the k evictions have a full
                # 2-group cycle of slack instead of gating the score matmuls
                if ib == 0 and g % 2 == 0 and g // 2 + 1 < NIB:
                    proj_block(g // 2 + 1)
                isl = slice(ib * IB, (ib + 1) * IB)
                ssg = ps.tile([P, JG, IB], f32, tag="score", bufs=2,
                              name="ssg")
                for t in range(JG):
                    jt = g * JG + t
                    for ci in range(CC):
                        nc.tensor.matmul(
                            ssg[:, t],
                            k_t[:, ci, jt * P:(jt + 1) * P],
                            q_t[:, ci, isl],
                            start=(ci == 0), stop=(ci == CC - 1))
                ssgs[ib, g] = ssg

            def expg(ib, g):
                ssg = ssgs.pop((ib, g))
                et = work.tile([P, JG, IB], f32r, tag="exp", bufs=6,
                               name="et")
                if ib == NIB - 1 and g == NGRP - 1:
                    # split so AV/fold on t0 start before t1's exp ends
                    for t in range(JG):
                        nc.scalar.activation(et[:, t], ssg[:, t], AF.Exp)
                else:
                    nc.scalar.activation(et[:], ssg[:], AF.Exp)
                ets[ib, g] = et

            # x + bo3 for the last i-block, precomputed on Pool during the
            # block's attention phase so the epilogue's residual adds are
            # plain tensor_adds splittable across DVE and Pool
            xb3_t = work.tile([P, CC, IB], f32, tag="xb3")

            def av(ib, g):
                if g == 0:
                    obd[ib] = [ps.tile([P, IB], f32, tag="ob", bufs=2,
                                       name=f"ob_{ib}_{co}")
                               for co in range(CC)]
                    if ib == NIB - 1:
                        isl7 = slice(ib * IB, (ib + 1) * IB)
                        for cc in range(CC):
                            nc.gpsimd.tensor_scalar_add(
                                xb3_t[:, cc], xt[:, cc, isl7],
                                bo3[:, cc:cc + 1])
                et = ets[ib, g]
                # final group of the last i-block: finish co1 entirely first
                # (co-major order) so the obs1 eviction (start of the co1
                # epilogue chain) begins two matmuls earlier
                if ib == NIB - 1 and g == NGRP - 1:
                    for co in (1, 0):
                        for t in range(JG):
                            jt = g * JG + t
                            nc.tensor.matmul(
                                obd[ib][co][:],
                                vp_t[:, jt, co * P:(co + 1) * P],
                                et[:, t],
                                start=(jt == 0), stop=(jt == NJT - 1))
                else:
                    for t in range(JG):
                        jt = g * JG + t
                        for co in range(CC):
                            nc.tensor.matmul(
                                obd[ib][co][:],
                                vp_t[:, jt, co * P:(co + 1) * P],
                                et[:, t],
                                start=(jt == 0), stop=(jt == NJT - 1))

            # denominator partial sums, collapsed to [P, IB]: Pool owns the
            # early groups (esb), DVE the late ones (esa); the final
            # group(s) feed the matmul fold directly so no elementwise add
            # sits on the i-block boundary. The fold lands in a "qk" PSUM
            # slot (free after phase 1), so it never collides with the
            # score rotation.
            def fold_mm(ib, es2, jgdim, start, stop):
                if ib not in smtd:
                    smtd[ib] = ps.tile([P, IB], f32, tag="qk", bufs=2,
                                       name="sml")
                if jgdim:
                    for t in range(JG):
                        nc.tensor.matmul(smtd[ib][:], ones_t[:], es2[:, t],
                                         start=(start and t == 0),
                                         stop=(stop and t == JG - 1))
                else:
                    nc.tensor.matmul(smtd[ib][:], ones_t[:], es2[:],
                                     start=start, stop=stop)

            def esum(ib, g):
                last_ib = ib == NIB - 1
                pool_n = 6 if last_ib else NGRP // 2
                # last i-block: only g15 folds directly (2 matmuls after the
                # split exp); g14's tiles merge into the DVE partial (slack
                # there), saving 2 PE matmuls at identical smt latency
                ndir = 1 if last_ib else 0
                et = ets[(ib, g)]
                if g == 0:
                    esd[ib] = (
                        work.tile([P, IB], f32r, tag="esum", bufs=2,
                                  name="esb"),
                        work.tile([P, IB], f32r, tag="esum", bufs=2,
                                  name="esa"))
                esb, esa = esd[ib]
                if ndir and g >= NGRP - ndir:
                    # t0's fold first (only needs exp15.t0); the esa fold
                    # (waiting the DVE merge) slots into the exp15.t1 wait
                    fold_mm(ib, et[:, 0], False, start=True, stop=False)
                    fold_mm(ib, esa, False, start=False, stop=False)
                    fold_mm(ib, et[:, 1], False, start=False, stop=True)
                    return
                if g == 0:
                    nc.gpsimd.tensor_add(esb[:], et[:, 0].bitcast(f32),
                                         et[:, 1].bitcast(f32))
                elif g < pool_n:
                    for t in range(JG):
                        nc.gpsimd.tensor_add(esb[:], esb[:].bitcast(f32),
                                             et[:, t].bitcast(f32))
                elif g == pool_n:
                    nc.vector.tensor_add(esa[:], et[:, 0].bitcast(f32),
                                         et[:, 1].bitcast(f32))
                else:
                    for t in range(JG):
                        nc.vector.tensor_add(esa[:], esa[:].bitcast(f32),
                                             et[:, t].bitcast(f32))
                if not last_ib and g == NGRP - 1:
                    # merge the Pool partial into the DVE one, then a single
                    # ones-matmul fold: 1 PE matmul/i-block instead of 4.
                    # tail(ib)'s divide has a full i-block of slack, so the
                    # later smt landing costs nothing here (the last i-block
                    # keeps the latency-optimized direct-fold path above).
                    nc.vector.tensor_add(esa[:], esa[:].bitcast(f32),
                                         esb[:].bitcast(f32))
                    fold_mm(ib, esa, False, start=True, stop=True)
                elif last_ib and g == NGRP - ndir - 1:
                    # last i-block: its DVE partial chain ends here, so merge
                    # the Pool partial in now -- g14 then folds a single
                    # combined tile, one PE matmul less ahead of the
                    # epilogue-critical smt
                    nc.vector.tensor_add(esa[:], esa[:].bitcast(f32),
                                         esb[:].bitcast(f32))

            def tail(ib):
                last_ib = ib == NIB - 1
                ob = obd.pop(ib)
                rec = work.tile([P, IB], f32, tag="rec", bufs=2, name="rec")
                smt = smtd.pop(ib)
                fins = [work.tile([P, IB], f32, tag="fin", bufs=4,
                                  name=f"fin{co}") for co in range(CC)]
                isl_f = slice(ib * IB, (ib + 1) * IB)
                if not last_ib:
                    # free the ob PSUM banks right away (obs copies off the
                    # ScalarE exp queue): the divide+residual then runs
                    # purely from SBUF, so Pool can take half
                    obs = [work.tile([P, IB], f32, tag="obs", bufs=3,
                                     name=f"obs{co}") for co in range(CC)]
                    for co in range(CC):
                        nc.vector.tensor_copy(obs[co][:], ob[co][:])
                    nc.vector.reciprocal(rec[:], smt[:])
                    nc.vector.tensor_mul(fins[0][:], obs[0][:], rec[:])
                    nc.vector.scalar_tensor_tensor(
                        fins[0][:], fins[0][:], bo3[:, 0:1],
                        xt[:, 0, isl_f], OP.add, OP.add)
                    nc.gpsimd.tensor_mul(fins[1][:], obs[1][:], rec[:])
                    # no scalar_tensor_tensor on Pool: finish co1 on DVE
                    nc.vector.scalar_tensor_tensor(
                        fins[1][:], fins[1][:], bo3[:, 1:2],
                        xt[:, 1, isl_f], OP.add, OP.add)
                    nc.sync.dma_start(out_r[:, 0, isl_f], fins[0][:])
                    nc.sync.dma_start(out_r[:, 1, isl_f], fins[1][:])
                    return
                # last i-block: nothing follows, so skip the ob evictions
                # (read PSUM directly), quarter everything, and issue each
                # quarter's DMAs (both channel chunks, on separate queues)
                # the moment its fin lands. The co1 path runs off an ACT
                # quarter-copy (Pool can't read PSUM); the +x+bias add uses
                # the xb3 tensor precomputed on Pool at the last i-block's
                # start, splitting the residual adds between DVE and Pool.
                obs1 = work.tile([P, IB], f32, tag="obs", bufs=3,
                                 name="obs1")
                HB = IB // 2
                halves = (slice(0, HB), slice(HB, IB))
                for hs in halves:
                    nc.scalar.activation(obs1[:, hs], ob[1][:, hs], AF.Copy)
                for hs in halves:
                    nc.vector.reciprocal(rec[:, hs], smt[:, hs])
                    # co1 muls on Pool (off obs1; Pool can't read PSUM) run
                    # concurrently with co0's DVE chain
                    nc.gpsimd.tensor_mul(fins[1][:, hs],
                                         obs1[:, hs], rec[:, hs])
                # co0's whole chain first: its full-tile DMA descriptor hits
                # the serial HWDGE early and its 728ns transfer overlaps the
                # co1 adds; co1 then drains as two half DMAs so the final
                # transfer gating the drain epilogue is only a half-tile
                for hs in halves:
                    hsl = slice(ib * IB + hs.start, ib * IB + hs.stop)
                    nc.vector.tensor_mul(fins[0][:, hs],
                                         ob[0][:, hs], rec[:, hs])
                    nc.vector.tensor_add(fins[0][:, hs], fins[0][:, hs],
                                         xb3_t[:, 0, hs])
                    nc.sync.dma_start(out_r[:, 0, hsl], fins[0][:, hs])
                for hs in halves:
                    hsl = slice(ib * IB + hs.start, ib * IB + hs.stop)
                    nc.gpsimd.tensor_add(fins[1][:, hs], fins[1][:, hs],
                                         xb3_t[:, 1, hs])
                    nc.scalar.dma_start(out_r[:, 1, hsl], fins[1][:, hs])

            proj_block(0)
            scores(0, 0)
            expg(0, 0)
            scores(0, 1)
            expg(0, 1)
            for ib in range(NIB):
                for g in range(NGRP):
                    # scores for g+2 run BEFORE av(g): the exp pipeline gains
                    # one AV-step of lead, so the first AV of each i-block no
                    # longer waits on a backed-up Activation queue
                    g2 = g + 2
                    if g2 < NGRP:
                        scores(ib, g2)
                        expg(ib, g2)
                    elif ib + 1 < NIB:
                        scores(ib + 1, g2 - NGRP)
                        expg(ib + 1, g2 - NGRP)
                    if ib == NIB - 1 and g == NGRP - 1:
                        # denominator fold matmuls BEFORE the last AV group:
                        # smt lands ~0.4us earlier, pulling the whole
                        # divide+residual epilogue chain forward
                        esum(ib, g)
                        av(ib, g)
                    else:
                        av(ib, g)
                        esum(ib, g)
                tail(ib)

    nc.compile()
    nc.m = get_hw_module(nc.m)
    return nc


def _get_nc():
    if "nc" not in _CACHE:
        _CACHE["nc"] = _build()
    return _CACHE["nc"]


def _prep_inputs(x, gn_scale, gn_bias, wq, bq, wk, bk, wv, bv, wo, bo):
    f = np.float32
    x = np.asarray(x, f)
    b = x.shape[0]
    scale = 1.0 / np.sqrt(np.float64(C))
    gns = np.asarray(gn_scale, np.float64)
    gnb = np.asarray(gn_bias, np.float64)
    wq64 = np.asarray(wq, np.float64)
    wk64 = np.asarray(wk, np.float64)
    wov64 = np.asarray(wo, np.float64) @ np.asarray(wv, np.float64)
    # gn_scale folds into the weight columns and w^T@gn_bias into the
    # biases host-side, leaving only the data-dependent rstd/mean GN
    # folds for the chip
    wqT = (wq64 * scale).T * gns[:, None]
    wkT = wk64.T * gns[:, None]
    wovT = wov64.T * gns[:, None]
    bq2 = ((np.asarray(bq, np.float64) + wq64 @ gnb) * scale).astype(f)
    bk2 = (np.asarray(bk, np.float64) + wk64 @ gnb).astype(f)
    bo2 = (np.asarray(bo, np.float64)
           + np.asarray(wo, np.float64) @ np.asarray(bv, np.float64)
           + wov64 @ gnb).astype(f)
    wstack = np.ascontiguousarray(
        np.stack([wqT, wkT, wovT]).astype(f))
    bstack = np.ascontiguousarray(np.stack(
        [bq2, bk2, bo2, np.asarray(gn_scale, f),
         np.asarray(gn_bias, f)]))

    gm = np.zeros((CC, P, G), f)
    for cc in range(CC):
        for p in range(P):
            gm[cc, p, (cc * P + p) // (C // G)] = 1.0
    gmT = np.ascontiguousarray(np.transpose(gm, (0, 2, 1)))

    shared = {"wstack": wstack, "bstack": bstack, "Gm": gm, "GmT": gmT}
    in_maps = []
    for i in range(b):
        m = dict(shared)
        m["x"] = np.ascontiguousarray(x[i].reshape(C, N))
        in_maps.append(m)
    return in_maps


def _run(in_maps, trace=False, trace_cores=None):
    from concourse import bass_utils
    nc = _get_nc()
    return bass_utils.run_bass_kernel_spmd(
        nc, in_maps, core_ids=list(range(len(in_maps))),
        trace=trace, trace_cores=trace_cores)


def kernel(x, gn_scale, gn_bias, wq, bq, wk, bk, wv, bv, wo, bo):
    in_maps = _prep_inputs(x, gn_scale, gn_bias, wq, bq, wk, bk, wv, bv, wo, bo)
    res = _run(in_maps)
    b = np.asarray(x).shape[0]
    out = np.stack([res.results[i]["out"].reshape(C, H, W) for i in range(b)])
    return out.astype(np.float32)

